# revision 10
# baseline (speedup 1.0000x reference)
"""3x3 NMS (maxpool + threshold + border) kernel for Trainium2, 8 NeuronCores.

Strategy:
  - Pure data parallel: 16 images -> 2 images per core on 8 cores.
  - Host zero-pads each image to H+2 rows so the kernel can load, per
    partition, R+2 consecutive rows (R=12 core rows + 1 halo row each
    side) with a single overlapping strided DMA. Partition p of a tile
    holds padded rows p*R .. p*R+R+1 (= image rows p*R-1 .. p*R+R).
  - The image is split into NT column tiles (2-col halos) to fit SBUF.
  - Per tile, 2 engine passes (split across gpsimd + vector so they
    pipeline; all exact max/compare, no arithmetic):
      1. gpsimd: t = max(x_up, x_dn)          (stock tensor_tensor)
      2. vector: mask[c] = (x[c] >= max(slide3(max(t,0.6))[c],
                                        slide3(x)[c])) as u8
         -- a hand-built 8-block DVE uop (ANT_NMS_SLIDE) that fuses the
         0.6 clamp, BOTH horizontal sliding-window max3 streams (over t
         and over the center row x) and the compare in ONE 1-elem/cycle
         pass, using delay-chain captures from CURR_ALU_OUT for the
         previous-element taps.
    x >= max(window incl. center, thr) is exactly
    (x == maxpool3x3(x)) & (x >= thr): bit-exact, no FP hazards.
  - Host: zero 10px border, np.nonzero -> (y, x) rows, exactly matching
    jnp.nonzero order (batch-major, then row, then col).
"""

import os
import sys

sys.path.insert(0, "/opt/trn_rl_repo")

import numpy as np

B, C, H, W = 16, 1, 1536, 1536
HP = H + 2                    # padded rows
N_CORES = 8
B_PER = B // N_CORES          # images per core
R = 12                        # rows per partition (128 * 12 = 1536)
NT = 2                        # column tiles per image
V = W // NT                   # valid (output) columns per tile
PAD = 2                       # column halo on each side
REP_THR = 0.6

_CACHE = {}
LAST_RESULTS = None


def _build_program():
    import concourse.bass as bass
    import concourse.bacc as bacc
    import concourse.mybir as mybir
    from concourse.tile import TileContext

    f32 = mybir.dt.float32
    u8 = mybir.dt.uint8
    MAX = mybir.AluOpType.max

    from concourse.dve_ops import DveOp, OPS, _COMPILE_CACHE
    from concourse.dve_spec import Spec, Src0, Src1, C0, maxx, lower
    from concourse.dve_uop import (
        DveOpSpec, InpSel, OutSel, OutPath, AluInp, AluOp, DelayInp,
    )
    from concourse.dve_ops import get_dve_sub_opcode


    def _mk_nms_uop(base_uop):
        u = base_uop  # copy of a lowered stock uop: keeps FSM/trigger/ctrl
        # input lanes: lane k surfaces as PREV_DELAY_{k-1} at block 0
        for i in range(len(u.inp)):
            u.inp_enable[i] = 0
        u.enable_input(InpSel.SRC_0, 1)    # chain 0: t   (vertical max)
        u.enable_input(InpSel.CONST_0, 2)  # chain 1: thr (consumed at dp0)
        u.enable_input(InpSel.SRC_1, 3)    # chain 2: x   (center row)
        for p in u.out_enable:
            u.out_enable[p] = 0
        u.enable_output(OutSel.ALU_OUT, OutPath.WR0_LO)
        u.require_inp0 = 1
        u.require_inp1 = 1

        dp = u.datapath_config
        for b in dp:
            b.op = AluOp.BYPASS
            b.alu_src0 = AluInp.PREV_ALU_OUT
            b.alu_src1 = AluInp.PREV_ALU_OUT
            b.alu_out_enable = 1
            b.swap_enable = 0
            b.alu_out_a_enable = 0
            b.alu_out_b_enable = 0
            for c in range(len(b.delay)):
                b.delay[c] = DelayInp.PREV_ALU_OUT
                b.delay_enable[c] = 0

        # Streams: in0 = t starting 1 col AFTER in1 = x. out(k) is the mask
        # for center col = x-col k-1; first valid k is 3 (row-start junk).
        # blk0: tc = max(t(k), thr); chain3 <- tc (prev-elem tap); pass x
        dp[0].enable_alu(AluOp.MAX, AluInp.PREV_DELAY_0, AluInp.PREV_DELAY_1)
        dp[0].enable_delay_from_src(DelayInp.CURR_ALU_OUT, 3)
        dp[0].pass_through_delay(2)
        # blk1: mt = max(tc(k), tc(k-1)); chain4 <- mt; pass x
        dp[1].enable_alu(AluOp.MAX, AluInp.PREV_ALU_OUT, AluInp.PREV_DELAY_3)
        dp[1].enable_delay_from_src(DelayInp.CURR_ALU_OUT, 4)
        dp[1].pass_through_delay(2)
        # blk2: A = max(tc(k-2..k)); chain5 <- A; pass x
        dp[2].enable_alu(AluOp.MAX, AluInp.PREV_ALU_OUT, AluInp.PREV_DELAY_4)
        dp[2].enable_delay_from_src(DelayInp.CURR_ALU_OUT, 5)
        dp[2].pass_through_delay(2)
        # blk3: bring x into the ALU path; chain1 <- x (tap, chain 1 is free
        # after blk0 consumed thr); carry A
        dp[3].enable_alu(AluOp.BYPASS, AluInp.PREV_DELAY_2)
        dp[3].enable_delay_from_src(DelayInp.CURR_ALU_OUT, 1)
        dp[3].pass_through_delay(5)
        # blk4: mx = max(x(k), x(k-1)); chain0 <- mx (chain 0 free after
        # blk0); carry x-tap + A
        dp[4].enable_alu(AluOp.MAX, AluInp.PREV_ALU_OUT, AluInp.PREV_DELAY_1)
        dp[4].enable_delay_from_src(DelayInp.CURR_ALU_OUT, 0)
        dp[4].pass_through_delay(1, 5)
        # blk5: Bx = max(x(k-2..k)); carry x-tap + A
        dp[5].enable_alu(AluOp.MAX, AluInp.PREV_ALU_OUT, AluInp.PREV_DELAY_0)
        dp[5].pass_through_delay(1, 5)
        # blk6: M = max(Bx(k), A(k-1)) -- the full 3x3 window max (clamped);
        # A(k-1) realigns the t stream (which leads x by 1 col); carry x-tap
        dp[6].enable_alu(AluOp.MAX, AluInp.PREV_ALU_OUT, AluInp.PREV_DELAY_5)
        dp[6].pass_through_delay(1)
        # blk7: out = (M <= x(k-1))  i.e. center >= full window max
        dp[7].enable_alu(AluOp.IS_LE, AluInp.PREV_ALU_OUT, AluInp.PREV_DELAY_1)
        return u


    _READY = {}


    def make_ops(ver="v3"):
        if _READY:
            return _READY["nms"]
        base = lower(Spec(body=maxx(maxx(Src0, C0), Src1)), ver=ver)
        assert len(base) == 1, len(base)

        nms_spec = Spec(body=maxx(maxx(Src0, C0), Src1))  # dummy; cache hit

        NMS = DveOp("ANT_NMS_SLIDE", nms_spec, subdim=False, uops_sha={})
        import concourse.dve_ops as dmod
        OPS.append(NMS)
        for i, op in enumerate(OPS):
            dmod._SUB_OPCODE_FOR_NAME[op.name] = dmod._CUSTOM_DVE_ROW_BASE + i
        dmod.CUSTOM_DVE_SPECS[NMS.name] = NMS.spec

        u = _mk_nms_uop(base[0])

        _COMPILE_CACHE[("ANT_NMS_SLIDE", ver)] = DveOpSpec(
            name="ANT_NMS_SLIDE", opcode=get_dve_sub_opcode("ANT_NMS_SLIDE"),
            uops=[u], rd1_en=True)
        _READY["nms"] = NMS
        return NMS

    NMS = make_ops()

    nc = bacc.Bacc()
    x_in = nc.declare_dram_parameter("x", [B_PER, HP, W], f32, isOutput=False)
    m_out = nc.declare_dram_parameter("mask", [B_PER, H, W], u8, isOutput=True)

    with TileContext(nc) as tc:
        with tc.tile_pool(name="pool", bufs=1) as pool:
            for img in range(B_PER):
                mi = m_out[img].rearrange("(p r) c -> p r c", r=R)
                for t in range(NT):
                    cs = max(t * V - PAD, 0)
                    ce = min(t * V + V + PAD, W)
                    WT = ce - cs
                    first = img == 0 and t == 0
                    last = img == B_PER - 1 and t == NT - 1

                    # X bufs=3: loads run up to two tiles ahead so the DMA
                    # engines never starve. T bufs=1: produced and consumed
                    # within one tile iteration on the same engine.
                    X = pool.tile([128, R + 2, WT], f32, tag="X", bufs=3,
                                  name=f"X_{img}_{t}")
                    T = pool.tile([128, R, WT], f32, tag="T", bufs=1,
                                  name=f"T_{img}_{t}")
                    MSK = pool.tile([128, R, V + 1], u8, tag="MSK", bufs=2,
                                    name=f"MSK_{img}_{t}")

                    # overlapping strided view: partition p, row slot j,
                    # col c  ->  x[img, p*R + j, cs + c]. The first tile is
                    # loaded in 4 row-slot chunks so the first vertical-max
                    # can start ~3x sooner (shorter pipeline ramp). Later
                    # tiles load only rows 1..12 from HBM; the 2 halo row
                    # slots (0 and 13) are duplicates of other partitions'
                    # rows and come from cheap SBUF->SBUF partition-shifted
                    # copies instead of re-reading HBM (-14% read traffic).
                    if first:
                        slot_chunks = [(0, 3), (3, 6), (6, 10), (10, R + 2)]
                        for s0, s1 in slot_chunks:
                            xi = bass.AP(x_in, img * HP * W + s0 * W + cs,
                                         [[R * W, 128], [W, s1 - s0], [1, WT]])
                            nc.sync.dma_start(out=X[:, s0:s1, :], in_=xi)
                    else:
                        xi = bass.AP(x_in, img * HP * W + W + cs,
                                     [[R * W, 128], [W, R], [1, WT]])
                        nc.sync.dma_start(out=X[:, 1:R + 1, :], in_=xi)
                        # image-edge halves of the halo (padded rows 0, HP-1)
                        nc.sync.dma_start(
                            out=X[0:1, 0:1, :],
                            in_=bass.AP(x_in, img * HP * W + cs,
                                        [[R * W, 1], [W, 1], [1, WT]]))
                        nc.sync.dma_start(
                            out=X[127:128, R + 1:R + 2, :],
                            in_=bass.AP(x_in, img * HP * W + (HP - 1) * W + cs,
                                        [[R * W, 1], [W, 1], [1, WT]]))
                        nc.sync.dma_start(out=X[1:128, 0:1, :],
                                          in_=X[0:127, R:R + 1, :])
                        nc.sync.dma_start(out=X[0:127, R + 1:R + 2, :],
                                          in_=X[1:128, 1:2, :])

                    # Pass 1: vertical neighbor max, rows only (center row
                    # joins inside the DVE op). Generic elementwise ops only
                    # compile on the DVE; gpsimd/scalar cannot take this.
                    # Pass 2: fused clamp + double sliding max3 + compare.
                    # out(k) = mask for col cs+k-1; k valid from 3. Both edge
                    # tiles stop short of the image edge -- the uncovered
                    # mask cols all fall in the host-zeroed border.
                    SL = V + 1
                    row_chunks = [(0, 1), (1, 4), (4, 8), (8, R)] if first \
                        else ([(0, 3), (3, 6), (6, 9), (9, R)] if last
                              else [(0, R)])
                    for r0, r1 in row_chunks:
                        nc.vector.tensor_tensor(
                            T[:, r0:r1, :], X[:, r0:r1, :],
                            X[:, r0 + 2:r1 + 2, :], MAX)
                    for r0, r1 in row_chunks:
                        nc.vector._custom_dve(
                            NMS,
                            out=MSK[:, r0:r1, 0:SL],
                            in0=T[:, r0:r1, 1:1 + SL],
                            in1=X[:, r0 + 1:r1 + 1, 0:SL],
                            s0=REP_THR)
                        if last:
                            # chunked store overlaps the remaining compute
                            nc.sync.dma_start(
                                out=mi[:, r0:r1, t * V:t * V + V - 2],
                                in_=MSK[:, r0:r1, 3:V + 1])
                    if t == 0:
                        # MSK col k = mask col k-1; cols 0,1 junk -> border
                        nc.sync.dma_start(out=mi[:, :, 0:V],
                                          in_=MSK[:, :, 1:V + 1])
                    elif not last:
                        # mask cols W-2, W-1 never written (both border;
                        # host reads whatever is in HBM and zeroes them)
                        nc.sync.dma_start(out=mi[:, :, t * V:t * V + V - 2],
                                          in_=MSK[:, :, 3:V + 1])
    nc.finalize()
    return nc


def _get_program():
    if "nc" not in _CACHE:
        _CACHE["nc"] = _build_program()
    return _CACHE["nc"]


def kernel(repeatability):
    global LAST_RESULTS
    from concourse.bass_utils import run_bass_kernel_spmd

    x = np.asarray(repeatability, dtype=np.float32).reshape(B, H, W)
    xp = np.zeros((B, HP, W), dtype=np.float32)
    xp[:, 1:H + 1, :] = x
    per_core = xp.reshape(N_CORES, B_PER, HP, W)
    in_maps = [{"x": np.ascontiguousarray(per_core[i])} for i in range(N_CORES)]

    nc = _get_program()
    res = run_bass_kernel_spmd(nc, in_maps, list(range(N_CORES)),
                               trace=bool(os.environ.get("NMS_TRACE")))
    LAST_RESULTS = res

    masks = np.stack([res.results[i]["mask"] for i in range(N_CORES)])
    mask_full = masks.reshape(B, C, H, W) != 0
    mask_full[:, :, :10, :] = False
    mask_full[:, :, -10:, :] = False
    mask_full[:, :, :, :10] = False
    mask_full[:, :, :, -10:] = False
    _, _, ys, xs = np.nonzero(mask_full)
    return np.stack([ys, xs]).astype(np.int32)


# revision 14
# speedup vs baseline: 1.8940x; 1.8940x over previous
"""3x3 NMS (maxpool + threshold + border) kernel for Trainium2, 8 NeuronCores.

Strategy:
  - Pure data parallel: 16 images -> 2 images per core on 8 cores.
  - Host zero-pads each image to H+2 rows so the kernel can load, per
    partition, R+2 consecutive rows (R=12 core rows + 1 halo row each
    side) with a single overlapping strided DMA. Partition p of a tile
    holds padded rows p*R .. p*R+R+1 (= image rows p*R-1 .. p*R+R).
  - The image is split into NT column tiles (2-col halos) to fit SBUF.
  - Per tile, 2 engine passes (split across gpsimd + vector so they
    pipeline; all exact max/compare, no arithmetic):
      1. gpsimd: t = max(x_up, x_dn)          (stock tensor_tensor)
      2. vector: mask[c] = (x[c] >= max(slide3(max(t,0.6))[c],
                                        slide3(x)[c])) as u8
         -- a hand-built 8-block DVE uop (ANT_NMS_SLIDE) that fuses the
         0.6 clamp, BOTH horizontal sliding-window max3 streams (over t
         and over the center row x) and the compare in ONE 1-elem/cycle
         pass, using delay-chain captures from CURR_ALU_OUT for the
         previous-element taps.
    x >= max(window incl. center, thr) is exactly
    (x == maxpool3x3(x)) & (x >= thr): bit-exact, no FP hazards.
  - Host: zero 10px border, np.nonzero -> (y, x) rows, exactly matching
    jnp.nonzero order (batch-major, then row, then col).
"""

import os
import sys

sys.path.insert(0, "/opt/trn_rl_repo")

import numpy as np

B, C, H, W = 16, 1, 1536, 1536
HP = H + 2                    # padded rows
N_CORES = 8
B_PER = B // N_CORES          # images per core
R = 12                        # rows per partition (128 * 12 = 1536)
NT = 2                        # column tiles per image
V = W // NT                   # valid (output) columns per tile
PAD = 2                       # column halo on each side
REP_THR = 0.6

_CACHE = {}
LAST_RESULTS = None


def _build_program():
    import concourse.bass as bass
    import concourse.bacc as bacc
    import concourse.mybir as mybir
    from concourse.tile import TileContext

    f32 = mybir.dt.float32
    u8 = mybir.dt.uint8
    MAX = mybir.AluOpType.max

    from concourse.dve_ops import DveOp, OPS, _COMPILE_CACHE
    from concourse.dve_spec import Spec, Src0, Src1, C0, maxx, lower
    from concourse.dve_uop import (
        DveOpSpec, InpSel, OutSel, OutPath, AluInp, AluOp, DelayInp,
    )
    from concourse.dve_ops import get_dve_sub_opcode


    def _mk_nms_uop(base_uop):
        u = base_uop  # copy of a lowered stock uop: keeps FSM/trigger/ctrl
        # input lanes: lane k surfaces as PREV_DELAY_{k-1} at block 0
        for i in range(len(u.inp)):
            u.inp_enable[i] = 0
        u.enable_input(InpSel.SRC_0, 1)    # chain 0: t   (vertical max)
        u.enable_input(InpSel.CONST_0, 2)  # chain 1: thr (consumed at dp0)
        u.enable_input(InpSel.SRC_1, 3)    # chain 2: x   (center row)
        for p in u.out_enable:
            u.out_enable[p] = 0
        u.enable_output(OutSel.ALU_OUT, OutPath.WR0_LO)
        u.require_inp0 = 1
        u.require_inp1 = 1

        dp = u.datapath_config
        for b in dp:
            b.op = AluOp.BYPASS
            b.alu_src0 = AluInp.PREV_ALU_OUT
            b.alu_src1 = AluInp.PREV_ALU_OUT
            b.alu_out_enable = 1
            b.swap_enable = 0
            b.alu_out_a_enable = 0
            b.alu_out_b_enable = 0
            for c in range(len(b.delay)):
                b.delay[c] = DelayInp.PREV_ALU_OUT
                b.delay_enable[c] = 0

        # Streams: in0 = t starting 1 col AFTER in1 = x. out(k) is the mask
        # for center col = x-col k-1; first valid k is 3 (row-start junk).
        # blk0: tc = max(t(k), thr); chain3 <- tc (prev-elem tap); pass x
        dp[0].enable_alu(AluOp.MAX, AluInp.PREV_DELAY_0, AluInp.PREV_DELAY_1)
        dp[0].enable_delay_from_src(DelayInp.CURR_ALU_OUT, 3)
        dp[0].pass_through_delay(2)
        # blk1: mt = max(tc(k), tc(k-1)); chain4 <- mt; pass x
        dp[1].enable_alu(AluOp.MAX, AluInp.PREV_ALU_OUT, AluInp.PREV_DELAY_3)
        dp[1].enable_delay_from_src(DelayInp.CURR_ALU_OUT, 4)
        dp[1].pass_through_delay(2)
        # blk2: A = max(tc(k-2..k)); chain5 <- A; pass x
        dp[2].enable_alu(AluOp.MAX, AluInp.PREV_ALU_OUT, AluInp.PREV_DELAY_4)
        dp[2].enable_delay_from_src(DelayInp.CURR_ALU_OUT, 5)
        dp[2].pass_through_delay(2)
        # blk3: bring x into the ALU path; chain1 <- x (tap, chain 1 is free
        # after blk0 consumed thr); carry A
        dp[3].enable_alu(AluOp.BYPASS, AluInp.PREV_DELAY_2)
        dp[3].enable_delay_from_src(DelayInp.CURR_ALU_OUT, 1)
        dp[3].pass_through_delay(5)
        # blk4: mx = max(x(k), x(k-1)); chain0 <- mx (chain 0 free after
        # blk0); carry x-tap + A
        dp[4].enable_alu(AluOp.MAX, AluInp.PREV_ALU_OUT, AluInp.PREV_DELAY_1)
        dp[4].enable_delay_from_src(DelayInp.CURR_ALU_OUT, 0)
        dp[4].pass_through_delay(1, 5)
        # blk5: Bx = max(x(k-2..k)); carry x-tap + A
        dp[5].enable_alu(AluOp.MAX, AluInp.PREV_ALU_OUT, AluInp.PREV_DELAY_0)
        dp[5].pass_through_delay(1, 5)
        # blk6: M = max(Bx(k), A(k-1)) -- the full 3x3 window max (clamped);
        # A(k-1) realigns the t stream (which leads x by 1 col); carry x-tap
        dp[6].enable_alu(AluOp.MAX, AluInp.PREV_ALU_OUT, AluInp.PREV_DELAY_5)
        dp[6].pass_through_delay(1)
        # blk7: out = (M <= x(k-1))  i.e. center >= full window max
        dp[7].enable_alu(AluOp.IS_LE, AluInp.PREV_ALU_OUT, AluInp.PREV_DELAY_1)
        return u


    _READY = {}


    def make_ops(ver="v3"):
        if _READY:
            return _READY["nms"]
        base = lower(Spec(body=maxx(maxx(Src0, C0), Src1)), ver=ver)
        assert len(base) == 1, len(base)

        nms_spec = Spec(body=maxx(maxx(Src0, C0), Src1))  # dummy; cache hit

        NMS = DveOp("ANT_NMS_SLIDE", nms_spec, subdim=False, uops_sha={})
        import concourse.dve_ops as dmod
        OPS.append(NMS)
        for i, op in enumerate(OPS):
            dmod._SUB_OPCODE_FOR_NAME[op.name] = dmod._CUSTOM_DVE_ROW_BASE + i
        dmod.CUSTOM_DVE_SPECS[NMS.name] = NMS.spec

        u = _mk_nms_uop(base[0])

        _COMPILE_CACHE[("ANT_NMS_SLIDE", ver)] = DveOpSpec(
            name="ANT_NMS_SLIDE", opcode=get_dve_sub_opcode("ANT_NMS_SLIDE"),
            uops=[u], rd1_en=True)
        _READY["nms"] = NMS
        return NMS

    NMS = make_ops()

    nc = bacc.Bacc()
    x_in = nc.declare_dram_parameter("x", [B_PER, HP, W], f32, isOutput=False)
    m_out = nc.declare_dram_parameter("mask", [B_PER, H, W], u8, isOutput=True)

    with TileContext(nc) as tc:
        with tc.tile_pool(name="pool", bufs=1) as pool:
            # tiny dummy transfer issued first: absorbs the DMA cold-start
            # latency during the engine-sync preamble so the first real load
            # runs at full speed
            WARM = pool.tile([128, 1, 1], f32, tag="WARM", bufs=1, name="warm")
            nc.sync.dma_start(
                out=WARM[:, :, :],
                in_=bass.AP(x_in, 0, [[W, 128], [1, 1], [1, 1]]))
            nc.vector.tensor_scalar_max(WARM[:, :, :], WARM[:, :, :], 0.0)
            for img in range(B_PER):
                mi = m_out[img].rearrange("(p r) c -> p r c", r=R)
                for t in range(NT):
                    cs = max(t * V - PAD, 0)
                    ce = min(t * V + V + PAD, W)
                    WT = ce - cs
                    first = img == 0 and t == 0
                    last = img == B_PER - 1 and t == NT - 1

                    # X bufs=3: loads run up to two tiles ahead so the DMA
                    # engines never starve. T bufs=1: produced and consumed
                    # within one tile iteration on the same engine.
                    X = pool.tile([128, R + 2, WT], f32, tag="X", bufs=3,
                                  name=f"X_{img}_{t}")
                    T = pool.tile([128, R, WT], f32, tag="T", bufs=1,
                                  name=f"T_{img}_{t}")
                    MSK = pool.tile([128, R, V + 1], u8, tag="MSK", bufs=2,
                                    name=f"MSK_{img}_{t}")

                    # overlapping strided view: partition p, row slot j,
                    # col c  ->  x[img, p*R + j, cs + c]. The first tile is
                    # loaded in 4 row-slot chunks so the first vertical-max
                    # can start ~3x sooner (shorter pipeline ramp).
                    slot_chunks = [(0, 4), (4, 8), (8, 11), (11, R + 2)] \
                        if first else [(0, R + 2)]
                    for s0, s1 in slot_chunks:
                        xi = bass.AP(x_in, img * HP * W + s0 * W + cs,
                                     [[R * W, 128], [W, s1 - s0], [1, WT]])
                        nc.sync.dma_start(out=X[:, s0:s1, :], in_=xi)

                    # Pass 1: vertical neighbor max, rows only (center row
                    # joins inside the DVE op). Generic elementwise ops only
                    # compile on the DVE; gpsimd/scalar cannot take this.
                    # Pass 2: fused clamp + double sliding max3 + compare.
                    # out(k) = mask for col cs+k-1; k valid from 3. Both edge
                    # tiles stop short of the image edge -- the uncovered
                    # mask cols all fall in the host-zeroed border.
                    SL = V + 1
                    row_chunks = [(0, 2), (2, 6), (6, 9), (9, R)] if first \
                        else ([(0, 3), (3, 6), (6, 9), (9, R)] if last
                              else [(0, R)])
                    for r0, r1 in row_chunks:
                        nc.vector.tensor_tensor(
                            T[:, r0:r1, :], X[:, r0:r1, :],
                            X[:, r0 + 2:r1 + 2, :], MAX)
                    for r0, r1 in row_chunks:
                        nc.vector._custom_dve(
                            NMS,
                            out=MSK[:, r0:r1, 0:SL],
                            in0=T[:, r0:r1, 1:1 + SL],
                            in1=X[:, r0 + 1:r1 + 1, 0:SL],
                            s0=REP_THR)
                        if last:
                            # chunked store overlaps the remaining compute
                            nc.sync.dma_start(
                                out=mi[:, r0:r1, t * V:t * V + V - 2],
                                in_=MSK[:, r0:r1, 3:V + 1])
                    if t == 0:
                        # MSK col k = mask col k-1; cols 0,1 junk -> border
                        nc.sync.dma_start(out=mi[:, :, 0:V],
                                          in_=MSK[:, :, 1:V + 1])
                    elif not last:
                        # mask cols W-2, W-1 never written (both border;
                        # host reads whatever is in HBM and zeroes them)
                        nc.sync.dma_start(out=mi[:, :, t * V:t * V + V - 2],
                                          in_=MSK[:, :, 3:V + 1])
    nc.finalize()
    return nc


def _get_program():
    if "nc" not in _CACHE:
        _CACHE["nc"] = _build_program()
    return _CACHE["nc"]


def kernel(repeatability):
    global LAST_RESULTS
    from concourse.bass_utils import run_bass_kernel_spmd

    x = np.asarray(repeatability, dtype=np.float32).reshape(B, H, W)
    xp = np.zeros((B, HP, W), dtype=np.float32)
    xp[:, 1:H + 1, :] = x
    per_core = xp.reshape(N_CORES, B_PER, HP, W)
    in_maps = [{"x": np.ascontiguousarray(per_core[i])} for i in range(N_CORES)]

    nc = _get_program()
    res = run_bass_kernel_spmd(nc, in_maps, list(range(N_CORES)),
                               trace=bool(os.environ.get("NMS_TRACE")))
    LAST_RESULTS = res

    masks = np.stack([res.results[i]["mask"] for i in range(N_CORES)])
    mask_full = masks.reshape(B, C, H, W) != 0
    mask_full[:, :, :10, :] = False
    mask_full[:, :, -10:, :] = False
    mask_full[:, :, :, :10] = False
    mask_full[:, :, :, -10:] = False
    _, _, ys, xs = np.nonzero(mask_full)
    return np.stack([ys, xs]).astype(np.int32)


# revision 17
# speedup vs baseline: 1.8976x; 1.0019x over previous
"""3x3 NMS (maxpool + threshold + border) kernel for Trainium2, 8 NeuronCores.

Strategy:
  - Pure data parallel: 16 images -> 2 images per core on 8 cores.
  - Host zero-pads each image to H+2 rows so the kernel can load, per
    partition, R+2 consecutive rows (R=12 core rows + 1 halo row each
    side) with a single overlapping strided DMA. Partition p of a tile
    holds padded rows p*R .. p*R+R+1 (= image rows p*R-1 .. p*R+R).
  - The image is split into NT column tiles (2-col halos) to fit SBUF.
  - Per tile, 2 engine passes (split across gpsimd + vector so they
    pipeline; all exact max/compare, no arithmetic):
      1. gpsimd: t = max(x_up, x_dn)          (stock tensor_tensor)
      2. vector: mask[c] = (x[c] >= max(slide3(max(t,0.6))[c],
                                        slide3(x)[c])) as u8
         -- a hand-built 8-block DVE uop (ANT_NMS_SLIDE) that fuses the
         0.6 clamp, BOTH horizontal sliding-window max3 streams (over t
         and over the center row x) and the compare in ONE 1-elem/cycle
         pass, using delay-chain captures from CURR_ALU_OUT for the
         previous-element taps.
    x >= max(window incl. center, thr) is exactly
    (x == maxpool3x3(x)) & (x >= thr): bit-exact, no FP hazards.
  - Host: zero 10px border, np.nonzero -> (y, x) rows, exactly matching
    jnp.nonzero order (batch-major, then row, then col).
"""

import os
import sys

sys.path.insert(0, "/opt/trn_rl_repo")

import numpy as np

B, C, H, W = 16, 1, 1536, 1536
HP = H + 2                    # padded rows
N_CORES = 8
B_PER = B // N_CORES          # images per core
R = 12                        # rows per partition (128 * 12 = 1536)
NT = 3                        # column tiles per image
V = W // NT                   # valid (output) columns per tile
PAD = 2                       # column halo on each side
REP_THR = 0.6

_CACHE = {}
LAST_RESULTS = None


def _build_program():
    import concourse.bass as bass
    import concourse.bacc as bacc
    import concourse.mybir as mybir
    from concourse.tile import TileContext

    f32 = mybir.dt.float32
    u8 = mybir.dt.uint8
    MAX = mybir.AluOpType.max

    from concourse.dve_ops import DveOp, OPS, _COMPILE_CACHE
    from concourse.dve_spec import Spec, Src0, Src1, C0, maxx, lower
    from concourse.dve_uop import (
        DveOpSpec, InpSel, OutSel, OutPath, AluInp, AluOp, DelayInp,
    )
    from concourse.dve_ops import get_dve_sub_opcode


    def _mk_nms_uop(base_uop):
        u = base_uop  # copy of a lowered stock uop: keeps FSM/trigger/ctrl
        # input lanes: lane k surfaces as PREV_DELAY_{k-1} at block 0
        for i in range(len(u.inp)):
            u.inp_enable[i] = 0
        u.enable_input(InpSel.SRC_0, 1)    # chain 0: t   (vertical max)
        u.enable_input(InpSel.CONST_0, 2)  # chain 1: thr (consumed at dp0)
        u.enable_input(InpSel.SRC_1, 3)    # chain 2: x   (center row)
        for p in u.out_enable:
            u.out_enable[p] = 0
        u.enable_output(OutSel.ALU_OUT, OutPath.WR0_LO)
        u.require_inp0 = 1
        u.require_inp1 = 1

        dp = u.datapath_config
        for b in dp:
            b.op = AluOp.BYPASS
            b.alu_src0 = AluInp.PREV_ALU_OUT
            b.alu_src1 = AluInp.PREV_ALU_OUT
            b.alu_out_enable = 1
            b.swap_enable = 0
            b.alu_out_a_enable = 0
            b.alu_out_b_enable = 0
            for c in range(len(b.delay)):
                b.delay[c] = DelayInp.PREV_ALU_OUT
                b.delay_enable[c] = 0

        # Streams: in0 = t starting 1 col AFTER in1 = x. out(k) is the mask
        # for center col = x-col k-1; first valid k is 3 (row-start junk).
        # blk0: tc = max(t(k), thr); chain3 <- tc (prev-elem tap); pass x
        dp[0].enable_alu(AluOp.MAX, AluInp.PREV_DELAY_0, AluInp.PREV_DELAY_1)
        dp[0].enable_delay_from_src(DelayInp.CURR_ALU_OUT, 3)
        dp[0].pass_through_delay(2)
        # blk1: mt = max(tc(k), tc(k-1)); chain4 <- mt; pass x
        dp[1].enable_alu(AluOp.MAX, AluInp.PREV_ALU_OUT, AluInp.PREV_DELAY_3)
        dp[1].enable_delay_from_src(DelayInp.CURR_ALU_OUT, 4)
        dp[1].pass_through_delay(2)
        # blk2: A = max(tc(k-2..k)); chain5 <- A; pass x
        dp[2].enable_alu(AluOp.MAX, AluInp.PREV_ALU_OUT, AluInp.PREV_DELAY_4)
        dp[2].enable_delay_from_src(DelayInp.CURR_ALU_OUT, 5)
        dp[2].pass_through_delay(2)
        # blk3: bring x into the ALU path; chain1 <- x (tap, chain 1 is free
        # after blk0 consumed thr); carry A
        dp[3].enable_alu(AluOp.BYPASS, AluInp.PREV_DELAY_2)
        dp[3].enable_delay_from_src(DelayInp.CURR_ALU_OUT, 1)
        dp[3].pass_through_delay(5)
        # blk4: mx = max(x(k), x(k-1)); chain0 <- mx (chain 0 free after
        # blk0); carry x-tap + A
        dp[4].enable_alu(AluOp.MAX, AluInp.PREV_ALU_OUT, AluInp.PREV_DELAY_1)
        dp[4].enable_delay_from_src(DelayInp.CURR_ALU_OUT, 0)
        dp[4].pass_through_delay(1, 5)
        # blk5: Bx = max(x(k-2..k)); carry x-tap + A
        dp[5].enable_alu(AluOp.MAX, AluInp.PREV_ALU_OUT, AluInp.PREV_DELAY_0)
        dp[5].pass_through_delay(1, 5)
        # blk6: M = max(Bx(k), A(k-1)) -- the full 3x3 window max (clamped);
        # A(k-1) realigns the t stream (which leads x by 1 col); carry x-tap
        dp[6].enable_alu(AluOp.MAX, AluInp.PREV_ALU_OUT, AluInp.PREV_DELAY_5)
        dp[6].pass_through_delay(1)
        # blk7: out = (M <= x(k-1))  i.e. center >= full window max
        dp[7].enable_alu(AluOp.IS_LE, AluInp.PREV_ALU_OUT, AluInp.PREV_DELAY_1)
        return u


    _READY = {}


    def make_ops(ver="v3"):
        if _READY:
            return _READY["nms"]
        base = lower(Spec(body=maxx(maxx(Src0, C0), Src1)), ver=ver)
        assert len(base) == 1, len(base)

        nms_spec = Spec(body=maxx(maxx(Src0, C0), Src1))  # dummy; cache hit

        NMS = DveOp("ANT_NMS_SLIDE", nms_spec, subdim=False, uops_sha={})
        import concourse.dve_ops as dmod
        OPS.append(NMS)
        for i, op in enumerate(OPS):
            dmod._SUB_OPCODE_FOR_NAME[op.name] = dmod._CUSTOM_DVE_ROW_BASE + i
        dmod.CUSTOM_DVE_SPECS[NMS.name] = NMS.spec

        u = _mk_nms_uop(base[0])

        _COMPILE_CACHE[("ANT_NMS_SLIDE", ver)] = DveOpSpec(
            name="ANT_NMS_SLIDE", opcode=get_dve_sub_opcode("ANT_NMS_SLIDE"),
            uops=[u], rd1_en=True)
        _READY["nms"] = NMS
        return NMS

    NMS = make_ops()

    nc = bacc.Bacc()
    x_in = nc.declare_dram_parameter("x", [B_PER, HP, W], f32, isOutput=False)
    m_out = nc.declare_dram_parameter("mask", [B_PER, H, W], u8, isOutput=True)

    with TileContext(nc) as tc:
        with tc.tile_pool(name="pool", bufs=1) as pool:
            # tiny dummy transfer issued first: absorbs the DMA cold-start
            # latency during the engine-sync preamble so the first real load
            # runs at full speed
            WARM = pool.tile([128, 1, 1], f32, tag="WARM", bufs=1, name="warm")
            nc.sync.dma_start(
                out=WARM[:, :, :],
                in_=bass.AP(x_in, 0, [[W, 128], [1, 1], [1, 1]]))
            nc.vector.tensor_scalar_max(WARM[:, :, :], WARM[:, :, :], 0.0)
            for img in range(B_PER):
                mi = m_out[img].rearrange("(p r) c -> p r c", r=R)
                for t in range(NT):
                    cs = max(t * V - PAD, 0)
                    ce = min(t * V + V + PAD, W)
                    WT = ce - cs
                    first = img == 0 and t == 0
                    last = img == B_PER - 1 and t == NT - 1

                    # X bufs=4: loads run up to three tiles ahead so the DMA
                    # engines never starve. T bufs=1: produced and consumed
                    # within one tile iteration on the same engine.
                    X = pool.tile([128, R + 2, WT], f32, tag="X", bufs=4,
                                  name=f"X_{img}_{t}")
                    T = pool.tile([128, R, WT], f32, tag="T", bufs=1,
                                  name=f"T_{img}_{t}")
                    MSK = pool.tile([128, R, V + 3], u8, tag="MSK", bufs=2,
                                    name=f"MSK_{img}_{t}")

                    # overlapping strided view: partition p, row slot j,
                    # col c  ->  x[img, p*R + j, cs + c]. The first tile is
                    # loaded in 4 row-slot chunks so the first vertical-max
                    # can start ~3x sooner (shorter pipeline ramp).
                    slot_chunks = [(0, 4), (4, 8), (8, 11), (11, R + 2)] \
                        if first else [(0, R + 2)]
                    for s0, s1 in slot_chunks:
                        xi = bass.AP(x_in, img * HP * W + s0 * W + cs,
                                     [[R * W, 128], [W, s1 - s0], [1, WT]])
                        nc.sync.dma_start(out=X[:, s0:s1, :], in_=xi)

                    # Pass 1: vertical neighbor max, rows only (center row
                    # joins inside the DVE op). Generic elementwise ops only
                    # compile on the DVE; gpsimd/scalar cannot take this.
                    # Pass 2: fused clamp + double sliding max3 + compare.
                    # out(k) = mask for col cs+k-1; k valid from 3. Both edge
                    # tiles stop short of the image edge -- the uncovered
                    # mask cols all fall in the host-zeroed border.
                    SL = V + 1 if (t == 0 or t == NT - 1) else V + 3
                    row_chunks = [(0, 2), (2, 6), (6, 9), (9, R)] if first \
                        else ([(0, 4), (4, 7), (7, 10), (10, R)] if last
                              else [(0, R)])
                    for r0, r1 in row_chunks:
                        nc.vector.tensor_tensor(
                            T[:, r0:r1, :], X[:, r0:r1, :],
                            X[:, r0 + 2:r1 + 2, :], MAX)
                    for r0, r1 in row_chunks:
                        nc.vector._custom_dve(
                            NMS,
                            out=MSK[:, r0:r1, 0:SL],
                            in0=T[:, r0:r1, 1:1 + SL],
                            in1=X[:, r0 + 1:r1 + 1, 0:SL],
                            s0=REP_THR)
                        if last:
                            # chunked store overlaps the remaining compute
                            nc.sync.dma_start(
                                out=mi[:, r0:r1, t * V:t * V + V - 2],
                                in_=MSK[:, r0:r1, 3:V + 1])
                    if t == 0:
                        # MSK col k = mask col k-1; cols 0,1 junk -> border
                        nc.sync.dma_start(out=mi[:, :, 0:V],
                                          in_=MSK[:, :, 1:V + 1])
                    elif t < NT - 1:
                        # middle tiles: MSK col k = mask col t*V-3+k
                        nc.sync.dma_start(out=mi[:, :, t * V:(t + 1) * V],
                                          in_=MSK[:, :, 3:V + 3])
                    elif not last:
                        # last column tile: mask cols W-2, W-1 never written
                        # (both border; host zeroes whatever HBM holds there)
                        nc.sync.dma_start(out=mi[:, :, t * V:t * V + V - 2],
                                          in_=MSK[:, :, 3:V + 1])
    nc.finalize()
    return nc


def _get_program():
    if "nc" not in _CACHE:
        _CACHE["nc"] = _build_program()
    return _CACHE["nc"]


def kernel(repeatability):
    global LAST_RESULTS
    from concourse.bass_utils import run_bass_kernel_spmd

    x = np.asarray(repeatability, dtype=np.float32).reshape(B, H, W)
    xp = np.zeros((B, HP, W), dtype=np.float32)
    xp[:, 1:H + 1, :] = x
    per_core = xp.reshape(N_CORES, B_PER, HP, W)
    in_maps = [{"x": np.ascontiguousarray(per_core[i])} for i in range(N_CORES)]

    nc = _get_program()
    res = run_bass_kernel_spmd(nc, in_maps, list(range(N_CORES)),
                               trace=bool(os.environ.get("NMS_TRACE")))
    LAST_RESULTS = res

    masks = np.stack([res.results[i]["mask"] for i in range(N_CORES)])
    mask_full = masks.reshape(B, C, H, W) != 0
    mask_full[:, :, :10, :] = False
    mask_full[:, :, -10:, :] = False
    mask_full[:, :, :, :10] = False
    mask_full[:, :, :, -10:] = False
    _, _, ys, xs = np.nonzero(mask_full)
    return np.stack([ys, xs]).astype(np.int32)
